# revision 44
# baseline (speedup 1.0000x reference)
import sys
import numpy as np

sys.path.insert(0, "/opt/trn_rl_repo")

import concourse.bass as bass  # noqa: E402
import concourse.bacc as bacc  # noqa: E402
import concourse.tile as tile  # noqa: E402
from concourse import mybir  # noqa: E402
from concourse.bass_utils import run_bass_kernel_spmd  # noqa: E402

import ml_dtypes  # noqa: E402

# Problem dims (hardcoded per spec)
N, T, V, C_IN, C_OUT, K, KT = 256, 2048, 9, 16, 3, 5, 9
F_IN = V * C_IN    # 144
F_OUT = V * C_OUT  # 27
N_CORES = 8
NPC = N // N_CORES  # 32 samples per core

F32 = mybir.dt.float32
F16 = mybir.dt.float16
F8 = mybir.dt.float8e3          # e3m4: 4 mantissa bits
NP_F8 = ml_dtypes.float8_e3m4

BLK = 120                       # t_out per conv block (halo 8 -> t_in 128)
NBLK = 18                       # 17 full + 1 partial (8 wide)
BPP = 6                         # blocks per panel
NPAN = 3                        # panels
PANW = BPP * BLK + 8            # 728 t_in cols per panel
TP = 4 + T + 120                # padded t cols in DRAM (head 4, tail 120)
SAMW = PANW * NPAN              # 2184 panel cols per sample (w/ overlaps)
NCHUNK = 4                      # sample-chunks per panel load
SPC = NPC // NCHUNK             # 8 samples per load chunk

_PROGRAM_CACHE = {}


DEFAULT_CFG = dict(
    psz_tags=2,      # GCN psum: distinct tags (1 or 2)
    psz_bufs=2,
    pso_bufs=1,
    pso_tags=3,      # 3 = per-o tags; 1 = shared rotating tag
    zt_bufs=6,
    osb_bufs=6,
    store_eng="sync",     # "gpsimd" | "scalar" | "sync"
    copy_eng="vector",    # engine for psum->zt copies: "vector"|"alt"
    conv_halves=True,     # conv matmuls per 16-sample half
    act_halves=False,     # per-half psum groups + activations (needs conv_halves)
    defer_stores=False,   # make stores depend on the last input load
    store_prio=None,      # override stores' scheduler priority (big = late)
    store_split=False,    # one store per (block, o) instead of per block
    panel_stores=2,       # first N panels store once per panel (merged)
    skew=True,            # emit conv/act/store of block j-1 after GCN of j
    skip_gcn=False, skip_conv=False, skip_act=False, skip_store=False,
)


LAST_LABELS = {}


def _lab(inst, label):
    try:
        LAST_LABELS[inst.ins.name] = label
    except Exception:
        pass
    return inst


def _build_program(cfg=None):
    cfg = {**DEFAULT_CFG, **(cfg or {})}
    LAST_LABELS.clear()
    nc = bacc.Bacc()

    xa = nc.declare_dram_parameter("xa", [NPC, 128, TP], F8, isOutput=False)
    xb = nc.declare_dram_parameter("xb", [NPC, 17, TP], F8, isOutput=False)
    w1 = nc.declare_dram_parameter("w1", [128, F_OUT], F16, isOutput=False)
    w2 = nc.declare_dram_parameter("w2", [17, F_OUT], F16, isOutput=False)
    bmat = nc.declare_dram_parameter("bmat", [128, 9 * BLK], F16, isOutput=False)
    btcn = nc.declare_dram_parameter("btcn", [BLK, C_OUT], F32, isOutput=False)
    # out[j, t', 288*o + 9*s + w]
    out = nc.declare_dram_parameter("out", [NBLK, BLK, NPC * F_OUT], F16,
                                    isOutput=True)

    with tile.TileContext(nc) as tc:
        with (
            tc.tile_pool(name="const", bufs=1) as cpool,
            tc.tile_pool(name="pa", bufs=3) as papool,
            tc.tile_pool(name="pb", bufs=3) as pbpool,
            tc.tile_pool(name="zt", bufs=cfg["zt_bufs"]) as ztpool,
            tc.tile_pool(name="osb", bufs=cfg["osb_bufs"]) as opool,
            tc.tile_pool(name="osbP", bufs=2) as opoolP,
            tc.tile_pool(name="psz", bufs=cfg["psz_bufs"],
                         space=bass.MemorySpace.PSUM) as psz_p,
            tc.tile_pool(name="pso", bufs=cfg["pso_bufs"],
                         space=bass.MemorySpace.PSUM) as pso_p,
        ):
            w1_sb = cpool.tile([128, F_OUT], F16, tag="w1")
            w2_sb = cpool.tile([17, F_OUT], F16, tag="w2")
            bmat_sb = cpool.tile([128, 9 * BLK], F16, tag="bmat")
            btcn_sb = cpool.tile([BLK, C_OUT], F32, tag="btcn")
            nc.gpsimd.dma_start(w1_sb[:], w1[:])
            nc.gpsimd.dma_start(w2_sb[:], w2[:])
            nc.gpsimd.dma_start(bmat_sb[:], bmat[:])
            nc.gpsimd.dma_start(btcn_sb[:], btcn[:])

            pa_ts, pb_ts = [], []
            for p in range(NPAN):
                pa_t = papool.tile([128, NPC * PANW], F8, tag="pa")
                pb_t = pbpool.tile([17, NPC * PANW], F8, tag="pb")
                pa_ts.append(pa_t)
                pb_ts.append(pb_t)
                pa_v = pa_t[:].rearrange("p (s c) -> p s c", s=NPC)
                pb_v = pb_t[:].rearrange("p (s c) -> p s c", s=NPC)
                c0 = 720 * p
                for ch in range(NCHUNK):
                    s0 = ch * SPC
                    _lab(nc.sync.dma_start(
                        pa_v[:, s0:s0 + SPC, :],
                        xa[s0:s0 + SPC, :, c0:c0 + PANW].transpose([1, 0, 2]),
                    ), f"loadA p{p}c{ch}")
                    last_load = _lab(nc.sync.dma_start(
                        pb_v[:, s0:s0 + SPC, :],
                        xb[s0:s0 + SPC, :, c0:c0 + PANW].transpose([1, 0, 2]),
                    ), f"loadB p{p}c{ch}")

            def emit_conv_o(j, wout, zt_v, o, osb=None):
                ptag = f"pso{o % cfg['pso_tags']}"
                pso = pso_p.tile([BLK, NPC * V], F32, tag=ptag,
                                 name=f"pso{o}_{j}")
                if cfg["conv_halves"]:
                    ah = cfg["act_halves"]
                    for half in range(2):
                        sl = slice(16 * half, 16 * half + 16)
                        for i in range(C_OUT):
                            kcol = BLK * (3 * i + o)
                            _lab(nc.tensor.matmul(
                                pso[0:wout, 144 * half:144 * half + 144],
                                bmat_sb[:, kcol:kcol + wout],
                                zt_v[:, sl, :, i],
                                start=(i == 0 and (ah or half == 0)),
                                stop=(i == 2 and (ah or half == 1)),
                            ), f"conv j{j}o{o}h{half}i{i}")
                        if ah:
                            c0 = 288 * o + 144 * half
                            _lab(nc.scalar.activation(
                                osb[0:wout, c0:c0 + 144],
                                pso[0:wout, 144 * half:144 * half + 144],
                                mybir.ActivationFunctionType.Lrelu,
                                bias=btcn_sb[0:wout, o:o + 1], alpha=0.01,
                            ), f"act j{j}o{o}h{half}")
                else:
                    for i in range(C_OUT):
                        kcol = BLK * (3 * i + o)
                        _lab(nc.tensor.matmul(
                            pso[0:wout, :],
                            bmat_sb[:, kcol:kcol + wout],
                            zt_v[:, :, :, i],
                            start=(i == 0), stop=(i == 2),
                        ), f"conv j{j}o{o}i{i}")
                return pso

            osbP_tiles = {}

            def emit_tail(j, wout, zt_t):
                p, b = j // BPP, j % BPP
                merged = p < cfg["panel_stores"]
                if merged:
                    if p not in osbP_tiles:
                        osbP_tiles[p] = opoolP.tile(
                            [BLK, BPP * NPC * F_OUT], F16, tag="osbP",
                            name=f"osbP{p}")
                    osb_full = osbP_tiles[p]
                    osb = osb_full[:, 864 * b:864 * b + 864]
                else:
                    osb = opool.tile([BLK, NPC * F_OUT], F16, tag="osb")
                zt_v = zt_t[:].rearrange("p (s w i) -> p s w i", w=V, i=C_OUT)
                for o in range(C_OUT):
                    if cfg["skip_conv"]:
                        break
                    pso = emit_conv_o(j, wout, zt_v, o, osb)
                    if cfg["skip_act"] or cfg["act_halves"]:
                        continue
                    _lab(nc.scalar.activation(
                        osb[0:wout, 288 * o:288 * o + 288], pso[0:wout, :],
                        mybir.ActivationFunctionType.Lrelu,
                        bias=btcn_sb[0:wout, o:o + 1], alpha=0.01,
                    ), f"act j{j}o{o}")
                if not (cfg["skip_store"] or cfg["skip_act"] or cfg["skip_conv"]):
                    seng = {"gpsimd": nc.gpsimd, "scalar": nc.scalar,
                            "sync": nc.sync}[cfg["store_eng"]]
                    if merged:
                        if b == BPP - 1:
                            src = osb_full[:].rearrange(
                                "p (b c) -> p b c", b=BPP)
                            dst = out[BPP * p:BPP * p + BPP].transpose([1, 0, 2])
                            peng = {"gpsimd": nc.gpsimd, "scalar": nc.scalar,
                                    "sync": nc.sync}[
                                cfg.get("pstore_eng") or cfg["store_eng"]]
                            _lab(peng.dma_start(dst, src), f"store P{p}")
                        return
                    if cfg["store_split"]:
                        sts = [
                            _lab(seng.dma_start(
                                out[j, 0:wout, 288 * o:288 * o + 288],
                                osb[0:wout, 288 * o:288 * o + 288]),
                                f"store j{j}o{o}")
                            for o in range(C_OUT)
                        ]
                    else:
                        sts = [_lab(seng.dma_start(out[j, 0:wout, :],
                                                   osb[0:wout, :]),
                                    f"store j{j}")]
                    if cfg["store_prio"] is not None:
                        for st in sts:
                            st.ins.bass_priority = cfg["store_prio"]
                    if cfg.get("store_wait_ts") and j < 2 * BPP:
                        for st in sts:
                            st.ins.bass_wait_until_ts = cfg["store_wait_ts"]
                    if cfg["defer_stores"]:
                        from concourse.tile_rust import add_dep_helper
                        for st in sts:
                            add_dep_helper(st.ins, last_load.ins, sync=True,
                                           reason="defer stores behind loads")

            pending = None
            for p in range(NPAN):
                pa_t, pb_t = pa_ts[p], pb_ts[p]
                for b in range(BPP):
                    j = BPP * p + b
                    wout = T - BLK * (NBLK - 1) if j == NBLK - 1 else BLK
                    zt_t = ztpool.tile([128, NPC * F_OUT], F16, tag="zt")
                    for half in range(2):
                        if cfg["skip_gcn"]:
                            break
                        ptag = f"psz{half % cfg['psz_tags']}"
                        psz = psz_p.tile([128, 16 * F_OUT], F32, tag=ptag)
                        for s16 in range(16):
                            s = 16 * half + s16
                            col0 = PANW * s + BLK * b
                            oc = F_OUT * s16
                            _lab(nc.tensor.matmul(
                                psz[:, oc:oc + F_OUT],
                                pa_t[:, col0:col0 + 128], w1_sb[:],
                                start=(s16 == 0), stop=False,
                            ), f"gcnA j{j}s{s}")
                            _lab(nc.tensor.matmul(
                                psz[:, oc:oc + F_OUT],
                                pb_t[:, col0:col0 + 128], w2_sb[:],
                                start=False, stop=(s16 == 15),
                            ), f"gcnB j{j}s{s}")
                        ceng = nc.vector
                        if cfg["copy_eng"] == "alt" and half == 1:
                            ceng = nc.scalar
                        _lab(ceng.tensor_copy(
                            zt_t[:, 432 * half:432 * half + 432], psz[:]),
                             f"copy j{j}h{half}")

                    if not cfg["skew"]:
                        emit_tail(j, wout, zt_t)
                    else:
                        if pending is not None:
                            emit_tail(*pending)
                        pending = (j, wout, zt_t)
            if pending is not None:
                emit_tail(*pending)

    nc.finalize()
    return nc


def _host_consts(A, W_gcn, b_gcn, W_tcn, b_tcn):
    A = np.asarray(A, np.float32)
    W_gcn = np.asarray(W_gcn, np.float32)
    b_gcn = np.asarray(b_gcn, np.float32)
    W_tcn = np.asarray(W_tcn, np.float32)
    b_tcn = np.asarray(b_tcn, np.float32)

    # W_eff[(v,c),(w,o)] = sum_k W_gcn[k,o,c] A[k,v,w]; z = x^T W_eff + b_eff
    W_eff = np.einsum("koc,kvw->vcwo", W_gcn, A).reshape(F_IN, F_OUT)
    b_eff = np.einsum("ko,kw->wo", b_gcn, A.sum(axis=1)).reshape(F_OUT)
    w1 = W_eff[:128].astype(np.float16)
    w2 = np.vstack([W_eff[128:], b_eff[None]]).astype(np.float16)

    # banded conv matrices: bmat[:, 120*(3i+o)+c][r] = W_tcn[o,i,8-(r-c)]
    bmat = np.zeros((128, 9 * BLK), np.float32)
    r = np.arange(128)[:, None]
    c = np.arange(BLK)[None, :]
    d = r - c
    mask = (d >= 0) & (d <= 8)
    dd = np.clip(d, 0, 8)
    for i in range(3):
        for o in range(3):
            blk = np.where(mask, W_tcn[o, i, 8 - dd, 0], 0.0)
            bmat[:, BLK * (3 * i + o):BLK * (3 * i + o + 1)] = blk
    bmat = bmat.astype(np.float16)

    btcn = np.tile(b_tcn[None, :], (BLK, 1)).astype(np.float32)
    return w1, w2, bmat, btcn


def _host_inputs(pose):
    # channel-major, fp8, padded cols: col u <-> t = u - 4
    x = np.ascontiguousarray(pose.transpose(0, 2, 1))  # [N, 144, T] f32
    xa = np.zeros((N, 128, TP), NP_F8)
    xb = np.zeros((N, 17, TP), NP_F8)
    xa[:, :, 4:4 + T] = x[:, :128].astype(NP_F8)
    xb[:, :16, 4:4 + T] = x[:, 128:].astype(NP_F8)
    xb[:, 16, 4:4 + T] = NP_F8(1.0)
    return xa, xb


def _run(inputs, **spmd_kwargs):
    pose = np.asarray(inputs["pose_feats"], np.float32)
    xa, xb = _host_inputs(pose)
    w1, w2, bmat, btcn = _host_consts(
        inputs["A"], inputs["W_gcn"], inputs["b_gcn"],
        inputs["W_tcn"], inputs["b_tcn"])

    if "prog" not in _PROGRAM_CACHE:
        _PROGRAM_CACHE["prog"] = _build_program()
    nc = _PROGRAM_CACHE["prog"]

    in_maps = []
    for i in range(N_CORES):
        sl = slice(i * NPC, (i + 1) * NPC)
        in_maps.append({
            "xa": xa[sl], "xb": xb[sl],
            "w1": w1, "w2": w2, "bmat": bmat, "btcn": btcn,
        })
    res = run_bass_kernel_spmd(nc, in_maps, list(range(N_CORES)), **spmd_kwargs)
    outs = [res.results[i]["out"] for i in range(N_CORES)]
    full = np.stack(outs, axis=0)              # [8, 18, 120, 864]
    # col = 288*o + 9*s + w ; y[core*32+s, 120*j+t', 3*w+o]
    full = full.reshape(N_CORES, NBLK, BLK, C_OUT, NPC, V).astype(np.float32)
    full = full.transpose(0, 4, 1, 2, 5, 3)    # [core, s, j, t', w, o]
    full = full.reshape(N, NBLK * BLK, F_OUT)[:, :T, :]
    return np.ascontiguousarray(full), res


def kernel(**inputs) -> np.ndarray:
    out, _ = _run(inputs)
    return out


# revision 45
# speedup vs baseline: 1.0117x; 1.0117x over previous
import sys
import numpy as np

sys.path.insert(0, "/opt/trn_rl_repo")

import concourse.bass as bass  # noqa: E402
import concourse.bacc as bacc  # noqa: E402
import concourse.tile as tile  # noqa: E402
from concourse import mybir  # noqa: E402
from concourse.bass_utils import run_bass_kernel_spmd  # noqa: E402

import ml_dtypes  # noqa: E402

# Problem dims (hardcoded per spec)
N, T, V, C_IN, C_OUT, K, KT = 256, 2048, 9, 16, 3, 5, 9
F_IN = V * C_IN    # 144
F_OUT = V * C_OUT  # 27
N_CORES = 8
NPC = N // N_CORES  # 32 samples per core

F32 = mybir.dt.float32
F16 = mybir.dt.float16
F8 = mybir.dt.float8e3          # e3m4: 4 mantissa bits
NP_F8 = ml_dtypes.float8_e3m4

BLK = 120                       # t_out per conv block (halo 8 -> t_in 128)
NBLK = 18                       # 17 full + 1 partial (8 wide)
BPP = 6                         # blocks per panel
NPAN = 3                        # panels
PANW = BPP * BLK + 8            # 728 t_in cols per panel
TP = 4 + T + 120                # padded t cols in DRAM (head 4, tail 120)
SAMW = PANW * NPAN              # 2184 panel cols per sample (w/ overlaps)
NCHUNK = 4                      # sample-chunks per panel load
SPC = NPC // NCHUNK             # 8 samples per load chunk

_PROGRAM_CACHE = {}


DEFAULT_CFG = dict(
    psz_tags=2,      # GCN psum: distinct tags (1 or 2)
    psz_bufs=2,
    pso_bufs=2,
    pso_tags=2,      # per-o modulo tags; conv psum = pso_tags * pso_bufs banks
    zt_bufs=6,
    osb_bufs=6,
    store_eng="sync",     # "gpsimd" | "scalar" | "sync"
    copy_eng="vector",    # engine for psum->zt copies: "vector"|"alt"
    conv_halves=True,     # conv matmuls per 16-sample half
    act_halves=False,     # per-half psum groups + activations (needs conv_halves)
    defer_stores=False,   # make stores depend on the last input load
    store_prio=None,      # override stores' scheduler priority (big = late)
    store_split=False,    # one store per (block, o) instead of per block
    panel_stores=2,       # first N panels store once per panel (merged)
    skew=True,            # emit conv/act/store of block j-1 after GCN of j
    skip_gcn=False, skip_conv=False, skip_act=False, skip_store=False,
)


LAST_LABELS = {}


def _lab(inst, label):
    try:
        LAST_LABELS[inst.ins.name] = label
    except Exception:
        pass
    return inst


def _build_program(cfg=None):
    cfg = {**DEFAULT_CFG, **(cfg or {})}
    LAST_LABELS.clear()
    nc = bacc.Bacc()

    xa = nc.declare_dram_parameter("xa", [NPC, 128, TP], F8, isOutput=False)
    xb = nc.declare_dram_parameter("xb", [NPC, 17, TP], F8, isOutput=False)
    w1 = nc.declare_dram_parameter("w1", [128, F_OUT], F16, isOutput=False)
    w2 = nc.declare_dram_parameter("w2", [17, F_OUT], F16, isOutput=False)
    bmat = nc.declare_dram_parameter("bmat", [128, 9 * BLK], F16, isOutput=False)
    btcn = nc.declare_dram_parameter("btcn", [BLK, C_OUT], F32, isOutput=False)
    # out[j, t', 288*o + 9*s + w]
    out = nc.declare_dram_parameter("out", [NBLK, BLK, NPC * F_OUT], F16,
                                    isOutput=True)

    with tile.TileContext(nc) as tc:
        with (
            tc.tile_pool(name="const", bufs=1) as cpool,
            tc.tile_pool(name="pa", bufs=3) as papool,
            tc.tile_pool(name="pb", bufs=3) as pbpool,
            tc.tile_pool(name="zt", bufs=cfg["zt_bufs"]) as ztpool,
            tc.tile_pool(name="osb", bufs=cfg["osb_bufs"]) as opool,
            tc.tile_pool(name="osbP", bufs=2) as opoolP,
            tc.tile_pool(name="psz", bufs=cfg["psz_bufs"],
                         space=bass.MemorySpace.PSUM) as psz_p,
            tc.tile_pool(name="pso", bufs=cfg["pso_bufs"],
                         space=bass.MemorySpace.PSUM) as pso_p,
        ):
            w1_sb = cpool.tile([128, F_OUT], F16, tag="w1")
            w2_sb = cpool.tile([17, F_OUT], F16, tag="w2")
            bmat_sb = cpool.tile([128, 9 * BLK], F16, tag="bmat")
            btcn_sb = cpool.tile([BLK, C_OUT], F32, tag="btcn")
            nc.gpsimd.dma_start(w1_sb[:], w1[:])
            nc.gpsimd.dma_start(w2_sb[:], w2[:])
            nc.gpsimd.dma_start(bmat_sb[:], bmat[:])
            nc.gpsimd.dma_start(btcn_sb[:], btcn[:])

            pa_ts, pb_ts = [], []
            for p in range(NPAN):
                pa_t = papool.tile([128, NPC * PANW], F8, tag="pa")
                pb_t = pbpool.tile([17, NPC * PANW], F8, tag="pb")
                pa_ts.append(pa_t)
                pb_ts.append(pb_t)
                pa_v = pa_t[:].rearrange("p (s c) -> p s c", s=NPC)
                pb_v = pb_t[:].rearrange("p (s c) -> p s c", s=NPC)
                c0 = 720 * p
                for ch in range(NCHUNK):
                    s0 = ch * SPC
                    _lab(nc.sync.dma_start(
                        pa_v[:, s0:s0 + SPC, :],
                        xa[s0:s0 + SPC, :, c0:c0 + PANW].transpose([1, 0, 2]),
                    ), f"loadA p{p}c{ch}")
                    last_load = _lab(nc.sync.dma_start(
                        pb_v[:, s0:s0 + SPC, :],
                        xb[s0:s0 + SPC, :, c0:c0 + PANW].transpose([1, 0, 2]),
                    ), f"loadB p{p}c{ch}")

            def emit_conv_o(j, wout, zt_v, o, osb=None):
                ptag = f"pso{o % cfg['pso_tags']}"
                pso = pso_p.tile([BLK, NPC * V], F32, tag=ptag,
                                 name=f"pso{o}_{j}")
                if cfg["conv_halves"]:
                    ah = cfg["act_halves"]
                    for half in range(2):
                        sl = slice(16 * half, 16 * half + 16)
                        for i in range(C_OUT):
                            kcol = BLK * (3 * i + o)
                            _lab(nc.tensor.matmul(
                                pso[0:wout, 144 * half:144 * half + 144],
                                bmat_sb[:, kcol:kcol + wout],
                                zt_v[:, sl, :, i],
                                start=(i == 0 and (ah or half == 0)),
                                stop=(i == 2 and (ah or half == 1)),
                            ), f"conv j{j}o{o}h{half}i{i}")
                        if ah:
                            c0 = 288 * o + 144 * half
                            _lab(nc.scalar.activation(
                                osb[0:wout, c0:c0 + 144],
                                pso[0:wout, 144 * half:144 * half + 144],
                                mybir.ActivationFunctionType.Lrelu,
                                bias=btcn_sb[0:wout, o:o + 1], alpha=0.01,
                            ), f"act j{j}o{o}h{half}")
                else:
                    for i in range(C_OUT):
                        kcol = BLK * (3 * i + o)
                        _lab(nc.tensor.matmul(
                            pso[0:wout, :],
                            bmat_sb[:, kcol:kcol + wout],
                            zt_v[:, :, :, i],
                            start=(i == 0), stop=(i == 2),
                        ), f"conv j{j}o{o}i{i}")
                return pso

            osbP_tiles = {}

            def emit_tail(j, wout, zt_t):
                p, b = j // BPP, j % BPP
                merged = p < cfg["panel_stores"]
                if merged:
                    if p not in osbP_tiles:
                        osbP_tiles[p] = opoolP.tile(
                            [BLK, BPP * NPC * F_OUT], F16, tag="osbP",
                            name=f"osbP{p}")
                    osb_full = osbP_tiles[p]
                    osb = osb_full[:, 864 * b:864 * b + 864]
                else:
                    osb = opool.tile([BLK, NPC * F_OUT], F16, tag="osb")
                zt_v = zt_t[:].rearrange("p (s w i) -> p s w i", w=V, i=C_OUT)
                for o in range(C_OUT):
                    if cfg["skip_conv"]:
                        break
                    pso = emit_conv_o(j, wout, zt_v, o, osb)
                    if cfg["skip_act"] or cfg["act_halves"]:
                        continue
                    _lab(nc.scalar.activation(
                        osb[0:wout, 288 * o:288 * o + 288], pso[0:wout, :],
                        mybir.ActivationFunctionType.Lrelu,
                        bias=btcn_sb[0:wout, o:o + 1], alpha=0.01,
                    ), f"act j{j}o{o}")
                if not (cfg["skip_store"] or cfg["skip_act"] or cfg["skip_conv"]):
                    seng = {"gpsimd": nc.gpsimd, "scalar": nc.scalar,
                            "sync": nc.sync}[cfg["store_eng"]]
                    if merged:
                        if b == BPP - 1:
                            src = osb_full[:].rearrange(
                                "p (b c) -> p b c", b=BPP)
                            dst = out[BPP * p:BPP * p + BPP].transpose([1, 0, 2])
                            peng = {"gpsimd": nc.gpsimd, "scalar": nc.scalar,
                                    "sync": nc.sync}[
                                cfg.get("pstore_eng") or cfg["store_eng"]]
                            _lab(peng.dma_start(dst, src), f"store P{p}")
                        return
                    if cfg["store_split"]:
                        sts = [
                            _lab(seng.dma_start(
                                out[j, 0:wout, 288 * o:288 * o + 288],
                                osb[0:wout, 288 * o:288 * o + 288]),
                                f"store j{j}o{o}")
                            for o in range(C_OUT)
                        ]
                    else:
                        sts = [_lab(seng.dma_start(out[j, 0:wout, :],
                                                   osb[0:wout, :]),
                                    f"store j{j}")]
                    if cfg["store_prio"] is not None:
                        for st in sts:
                            st.ins.bass_priority = cfg["store_prio"]
                    if cfg.get("store_wait_ts") and j < 2 * BPP:
                        for st in sts:
                            st.ins.bass_wait_until_ts = cfg["store_wait_ts"]
                    if cfg["defer_stores"]:
                        from concourse.tile_rust import add_dep_helper
                        for st in sts:
                            add_dep_helper(st.ins, last_load.ins, sync=True,
                                           reason="defer stores behind loads")

            pending = None
            for p in range(NPAN):
                pa_t, pb_t = pa_ts[p], pb_ts[p]
                for b in range(BPP):
                    j = BPP * p + b
                    wout = T - BLK * (NBLK - 1) if j == NBLK - 1 else BLK
                    zt_t = ztpool.tile([128, NPC * F_OUT], F16, tag="zt")
                    for half in range(2):
                        if cfg["skip_gcn"]:
                            break
                        ptag = f"psz{half % cfg['psz_tags']}"
                        psz = psz_p.tile([128, 16 * F_OUT], F32, tag=ptag)
                        for s16 in range(16):
                            s = 16 * half + s16
                            col0 = PANW * s + BLK * b
                            oc = F_OUT * s16
                            _lab(nc.tensor.matmul(
                                psz[:, oc:oc + F_OUT],
                                pa_t[:, col0:col0 + 128], w1_sb[:],
                                start=(s16 == 0), stop=False,
                            ), f"gcnA j{j}s{s}")
                            _lab(nc.tensor.matmul(
                                psz[:, oc:oc + F_OUT],
                                pb_t[:, col0:col0 + 128], w2_sb[:],
                                start=False, stop=(s16 == 15),
                            ), f"gcnB j{j}s{s}")
                        ceng = nc.vector
                        if cfg["copy_eng"] == "alt" and half == 1:
                            ceng = nc.scalar
                        _lab(ceng.tensor_copy(
                            zt_t[:, 432 * half:432 * half + 432], psz[:]),
                             f"copy j{j}h{half}")

                    if not cfg["skew"]:
                        emit_tail(j, wout, zt_t)
                    else:
                        if pending is not None:
                            emit_tail(*pending)
                        pending = (j, wout, zt_t)
            if pending is not None:
                emit_tail(*pending)

    nc.finalize()
    return nc


def _host_consts(A, W_gcn, b_gcn, W_tcn, b_tcn):
    A = np.asarray(A, np.float32)
    W_gcn = np.asarray(W_gcn, np.float32)
    b_gcn = np.asarray(b_gcn, np.float32)
    W_tcn = np.asarray(W_tcn, np.float32)
    b_tcn = np.asarray(b_tcn, np.float32)

    # W_eff[(v,c),(w,o)] = sum_k W_gcn[k,o,c] A[k,v,w]; z = x^T W_eff + b_eff
    W_eff = np.einsum("koc,kvw->vcwo", W_gcn, A).reshape(F_IN, F_OUT)
    b_eff = np.einsum("ko,kw->wo", b_gcn, A.sum(axis=1)).reshape(F_OUT)
    w1 = W_eff[:128].astype(np.float16)
    w2 = np.vstack([W_eff[128:], b_eff[None]]).astype(np.float16)

    # banded conv matrices: bmat[:, 120*(3i+o)+c][r] = W_tcn[o,i,8-(r-c)]
    bmat = np.zeros((128, 9 * BLK), np.float32)
    r = np.arange(128)[:, None]
    c = np.arange(BLK)[None, :]
    d = r - c
    mask = (d >= 0) & (d <= 8)
    dd = np.clip(d, 0, 8)
    for i in range(3):
        for o in range(3):
            blk = np.where(mask, W_tcn[o, i, 8 - dd, 0], 0.0)
            bmat[:, BLK * (3 * i + o):BLK * (3 * i + o + 1)] = blk
    bmat = bmat.astype(np.float16)

    btcn = np.tile(b_tcn[None, :], (BLK, 1)).astype(np.float32)
    return w1, w2, bmat, btcn


def _host_inputs(pose):
    # channel-major, fp8, padded cols: col u <-> t = u - 4
    x = np.ascontiguousarray(pose.transpose(0, 2, 1))  # [N, 144, T] f32
    xa = np.zeros((N, 128, TP), NP_F8)
    xb = np.zeros((N, 17, TP), NP_F8)
    xa[:, :, 4:4 + T] = x[:, :128].astype(NP_F8)
    xb[:, :16, 4:4 + T] = x[:, 128:].astype(NP_F8)
    xb[:, 16, 4:4 + T] = NP_F8(1.0)
    return xa, xb


def _run(inputs, **spmd_kwargs):
    pose = np.asarray(inputs["pose_feats"], np.float32)
    xa, xb = _host_inputs(pose)
    w1, w2, bmat, btcn = _host_consts(
        inputs["A"], inputs["W_gcn"], inputs["b_gcn"],
        inputs["W_tcn"], inputs["b_tcn"])

    if "prog" not in _PROGRAM_CACHE:
        _PROGRAM_CACHE["prog"] = _build_program()
    nc = _PROGRAM_CACHE["prog"]

    in_maps = []
    for i in range(N_CORES):
        sl = slice(i * NPC, (i + 1) * NPC)
        in_maps.append({
            "xa": xa[sl], "xb": xb[sl],
            "w1": w1, "w2": w2, "bmat": bmat, "btcn": btcn,
        })
    res = run_bass_kernel_spmd(nc, in_maps, list(range(N_CORES)), **spmd_kwargs)
    outs = [res.results[i]["out"] for i in range(N_CORES)]
    full = np.stack(outs, axis=0)              # [8, 18, 120, 864]
    # col = 288*o + 9*s + w ; y[core*32+s, 120*j+t', 3*w+o]
    full = full.reshape(N_CORES, NBLK, BLK, C_OUT, NPC, V).astype(np.float32)
    full = full.transpose(0, 4, 1, 2, 5, 3)    # [core, s, j, t', w, o]
    full = full.reshape(N, NBLK * BLK, F_OUT)[:, :T, :]
    return np.ascontiguousarray(full), res


def kernel(**inputs) -> np.ndarray:
    out, _ = _run(inputs)
    return out
